# revision 1
# baseline (speedup 1.0000x reference)
"""AutoRegressive LSTM kernel for Trainium2 (8 NeuronCores, data-parallel).

Reference computation (B=65536, T=96, F=2, HIDDEN=64, out_steps=24):
  - warmup: run an LSTM cell over 96 timesteps of the input sequence
  - decode: 24 autoregressive steps, feeding pred = h @ Wd + bd back as x

Design:
  - Pure data parallel: batch is sharded 8192 per core.
  - Gate-major on-chip layout: all state tensors are [dim, batch] so the
    recurrent matmul z = [U;W].T @ [h;x] streams batch columns through the
    PE with the tiny weights stationary (K=66 contraction).
  - The input sequence is pre-transposed on the host to [192, B] so x_t^T
    slices are contiguous rows; per-step x_t is DMA'd into partitions 64:66
    of the state tile H = [h; x].
  - Each gate gets its own M=64 matmul so every PSUM tile sits at base
    partition 0 (walrus requires equal base partitions for SBUF operands);
    sigmoid/tanh run on the scalar (ACT) engine with bias folded in,
    cell/hidden updates on the vector engine (DVE).
  - Batch is processed in blocks of 2048; tile pools keep 2 blocks in
    flight so PE/ACT/DVE work on different blocks overlaps.
"""

import sys

sys.path.insert(0, "/opt/trn_rl_repo")

import numpy as np

import concourse.bass as bass
import concourse.tile as tile
from concourse import bacc, mybir
from concourse.bass_utils import run_bass_kernel_spmd

HIDDEN = 64
T_WARM = 96
OUT_STEPS = 24
N_CORES = 8
B_TOTAL = 65536
B_CORE = B_TOTAL // N_CORES  # 8192
BT = 2048  # batch block per set of tiles
NBLK = B_CORE // BT  # 4
F32 = mybir.dt.float32
AF = mybir.ActivationFunctionType

_CACHE = {}


def _build_nc(reps=1):
    nc = bacc.Bacc("TRN2")
    xT = nc.declare_dram_parameter("xT", [2 * T_WARM, B_CORE], F32, isOutput=False)
    WU = nc.declare_dram_parameter("WU", [66, 256], F32, isOutput=False)
    Wd = nc.declare_dram_parameter("Wd", [HIDDEN, HIDDEN], F32, isOutput=False)
    bias4 = nc.declare_dram_parameter("bias4", [64, 4], F32, isOutput=False)
    bd = nc.declare_dram_parameter("bd", [HIDDEN, 1], F32, isOutput=False)
    out = nc.declare_dram_parameter("out", [2 * OUT_STEPS, B_CORE], F32, isOutput=True)

    with tile.TileContext(nc) as tc:
        with (
            tc.tile_pool(name="wpool", bufs=1) as wpool,
            tc.tile_pool(name="state", bufs=2) as state,
            tc.tile_pool(name="actp", bufs=2) as actp,
            tc.tile_pool(name="ps", bufs=4, space=bass.MemorySpace.PSUM) as ps,
        ):
            wu = wpool.tile([66, 256], F32, tag="wu")
            nc.sync.dma_start(wu[:], WU[:])
            wd = wpool.tile([HIDDEN, HIDDEN], F32, tag="wd")
            nc.sync.dma_start(wd[:], Wd[:])
            bias_t = wpool.tile([64, 4], F32, tag="bias4")
            nc.sync.dma_start(bias_t[:], bias4[:])
            bd_t = wpool.tile([HIDDEN, 1], F32, tag="bd")
            nc.sync.dma_start(bd_t[:], bd[:])
            # dependency-free warmup activations: the ACT table-load walrus
            # inserts before the first Sigmoid/Tanh lands here, not on a
            # real instruction that already carries 2 waits
            warm = wpool.tile([64, 4], F32, tag="warm")
            warm2 = wpool.tile([64, 4], F32, tag="warm2")
            nc.scalar.activation(warm[:], warm[:], AF.Sigmoid)
            nc.scalar.activation(warm2[:], warm2[:], AF.Tanh, bias=bias_t[:, 2:3])
            b_i, b_f, b_g, b_o = (bias_t[:, k : k + 1] for k in range(4))

            for blk_rep in range(NBLK * reps):
                blk = blk_rep % NBLK
                c0 = blk * BT

                H = state.tile([66, BT], F32, tag="H")
                C = state.tile([64, BT], F32, tag="C")
                nc.vector.memset(C[:], 0.0)

                SI = actp.tile([64, BT], F32, tag="si")
                SF = actp.tile([64, BT], F32, tag="sf")
                TG = actp.tile([64, BT], F32, tag="tg")
                # SO/TC carry 2 extra partitions: SO[64:66] holds the next
                # step's x (DMA'd from DRAM), TC[64:66] is constant 1.0, so
                # the h-update H[0:66] = SO*TC also moves x into H with the
                # same DVE instruction (keeps matmul wait count at 2).
                SO = actp.tile([66, BT], F32, tag="so")
                TC = actp.tile([66, BT], F32, tag="tc")
                T1 = actp.tile([64, BT], F32, tag="t1")
                pred_c = actp.tile([64, BT], F32, tag="predc")
                xsn = actp.tile([66, 4], F32, tag="xsn")
                ascr = actp.tile([1, 4], F32, tag="ascr")
                # step 0 reads x0 from X0 (h=0 so gates are W.T @ x0 + b);
                # H itself is first written by step 0's h-update
                X0 = state.tile([66, BT], F32, tag="X0")
                nc.sync.dma_start(X0[64:66, :], xT[0:2, c0 : c0 + BT])
                nc.vector.memset(TC[64:66, :], 1.0)

                def dummy_mm(ptile):
                    # 1-column matmul on static weights: absorbs the psum
                    # WAR (ACT) + PE-self waits so the following real matmul
                    # only needs the H dependency (hardware allows 2 waits)
                    nc.tensor.matmul(
                        ptile[:, 0:1], wu[:, 0:64], wu[:, 0:1],
                        start=True, stop=True, skip_group_check=True,
                    )

                def cell_step(with_x=False, first=False):
                    # one LSTM cell update for the whole block; x is in H[64:66]
                    gates = ((SI, 0, AF.Sigmoid, b_i), (SF, 64, AF.Sigmoid, b_f),
                             (TG, 128, AF.Tanh, b_g), (SO, 192, AF.Sigmoid, b_o))
                    if not first:
                        # scalar-engine sniff of H: observes the previous
                        # h-op's DVE tick so gate ACTs stay at <=2 waits
                        nc.scalar.copy(ascr[0:1, 0:1], H[0:1, 0:1])
                    for h0 in range(0, BT, 1024):
                        for dst, gc, fn, bb in gates:
                            pg = ps.tile([64, 1024], F32, tag="ps")
                            if h0 == 0 and gc == 0:
                                dummy_mm(pg)
                            if first:
                                lhs, rhs = wu[64:66, gc : gc + 64], X0[64:66, :]
                            else:
                                lhs, rhs = wu[:, gc : gc + 64], H[:, :]
                            nc.tensor.matmul(
                                pg[:, 0:512], lhs, rhs[:, h0 : h0 + 512],
                                start=True, stop=True,
                            )
                            nc.tensor.matmul(
                                pg[:, 512:1024], lhs, rhs[:, h0 + 512 : h0 + 1024],
                                start=True, stop=True,
                            )
                            nc.scalar.activation(
                                dst[0:64, h0 : h0 + 1024], pg[:], fn, bias=bb
                            )
                    nc.vector.tensor_mul(T1[:], SI[:], TG[:])
                    nc.vector.tensor_mul(C[:], SF[:], C[:])
                    nc.vector.tensor_add(C[:], C[:], T1[:])
                    nc.scalar.activation(TC[0:64, :], C[:], AF.Tanh)
                    if with_x:
                        nc.vector.tensor_mul(
                            xsn[64:66, 0:1], SO[64:66, 0:1], SO[64:66, 0:1]
                        )
                        nc.vector.tensor_mul(H[0:66, :], SO[0:66, :], TC[0:66, :])
                    else:
                        nc.vector.tensor_mul(H[0:64, :], SO[0:64, :], TC[0:64, :])

                def load_x(t_next):
                    src = xT[2 * t_next : 2 * t_next + 2, c0 : c0 + BT]
                    nc.sync.dma_start(SO[64:66, :], src)

                def pred_step(s):
                    # pred = h @ Wd + bd, written into H[64:66] (the x slot
                    # for the next decode step) and streamed out to DRAM
                    for h0 in range(0, BT, 1024):
                        # Wd is zero-padded to [64,64] on the host so pred
                        # psum tiles match the gate tiles exactly (uniform
                        # slot recycling keeps wait counts at 2)
                        pp = ps.tile([64, 1024], F32, tag="ps")
                        if h0 == 0:
                            dummy_mm(pp)
                        nc.tensor.matmul(
                            pp[:, 0:512], wd[:], H[0:64, h0 : h0 + 512],
                            start=True, stop=True,
                        )
                        nc.tensor.matmul(
                            pp[:, 512:1024], wd[:], H[0:64, h0 + 512 : h0 + 1024],
                            start=True, stop=True,
                        )
                        nc.vector.tensor_scalar_add(
                            pred_c[:, h0 : h0 + 1024], pp[:], bd_t[:]
                        )
                    nc.sync.dma_start(out[2 * s : 2 * s + 2, c0 : c0 + BT], pred_c[0:2, :])
                    if s + 1 < OUT_STEPS:
                        nc.vector.tensor_copy(H[64:66, :], pred_c[0:2, :])

                for t in range(T_WARM):
                    if t + 1 < T_WARM:
                        load_x(t + 1)
                        cell_step(with_x=True, first=(t == 0))
                    else:
                        cell_step(with_x=False)

                pred_step(0)
                for s in range(1, OUT_STEPS):
                    cell_step(with_x=False)
                    pred_step(s)
    nc.compile()
    return nc


def _get_nc():
    if "nc" not in _CACHE:
        _CACHE["nc"] = _build_nc()
    return _CACHE["nc"]


def _prep_in_maps(inputs, W, U, b, Wd, bd):
    inputs = np.asarray(inputs, dtype=np.float32)
    W = np.asarray(W, dtype=np.float32)
    U = np.asarray(U, dtype=np.float32)
    b = np.asarray(b, dtype=np.float32)
    Wd = np.asarray(Wd, dtype=np.float32)
    bd = np.asarray(bd, dtype=np.float32)

    B = inputs.shape[0]
    # [B, T, F] -> [T*F, B] so x_t^T rows are contiguous
    xT_full = np.ascontiguousarray(inputs.reshape(B, 2 * T_WARM).T)

    WU = np.ascontiguousarray(np.concatenate([U, W], axis=0))  # [66, 256]
    bias4 = np.ascontiguousarray(b.reshape(4, 64).T)  # [64, 4] cols = i,f,g,o
    bd_c = np.zeros((64, 1), np.float32)
    bd_c[0:2, 0] = bd
    Wd_c = np.zeros((64, 64), np.float32)
    Wd_c[:, 0:2] = Wd

    in_maps = []
    for i in range(N_CORES):
        sl = slice(i * B_CORE, (i + 1) * B_CORE)
        in_maps.append(
            {
                "xT": np.ascontiguousarray(xT_full[:, sl]),
                "WU": WU,
                "Wd": Wd_c,
                "bias4": bias4,
                "bd": bd_c,
            }
        )
    return in_maps


def _run(in_maps, trace=False, **kw):
    nc = _get_nc()
    res = run_bass_kernel_spmd(nc, in_maps, list(range(N_CORES)), trace=trace, **kw)
    return res


def kernel(inputs, W, U, b, Wd, bd, out_steps):
    assert int(out_steps) == OUT_STEPS
    in_maps = _prep_in_maps(inputs, W, U, b, Wd, bd)
    res = _run(in_maps)
    outs = []
    for i in range(N_CORES):
        o = np.asarray(res.results[i]["out"])  # [48, 8192]
        outs.append(o.T.reshape(B_CORE, OUT_STEPS, 2))
    return np.concatenate(outs, axis=0)

